# revision 12
# baseline (speedup 1.0000x reference)
"""GaussSynthesis Trainium2 kernel.

reference:  Y_ri = h @ weight            [B,S,2n]  (n=256 freqs)
            full spectrum bins 1..n = Y, rest zero
            out  = irfft(full, n=V)      [B,S,V]   (V=50257, odd)

Closed form (V odd, only bins 1..n nonzero):
    out[t]   = (2/V) * sum_k ( R_k cos(2 pi k t / V) - I_k sin(2 pi k t / V) )
    out[V-t] = (2/V) * sum_k ( R_k cos(2 pi k t / V) + I_k sin(2 pi k t / V) )
so only t = 0..(V-1)/2 = 25128 must be computed: two matmuls against a
cos/sin basis, then a sum/difference combine covers the full output.

Device plan (SPMD over 8 cores, 512 rows each, no collectives):
  stage 1: Y^T[f, r] = (scale*W)^T @ h^T   (fp16 inputs, fp32 psum -> fp16)
  stage 2: per 512-wide t-chunk: psum_c = R^T-part @ cos-chunk,
           psum_s = I^T-part @ sin-chunk (2 accumulating matmuls each),
           lo = c - s, hi = c + s  (ScalarE copies + VectorE tensor_tensor),
           DMA lo/hi to DRAM.
Host: builds the fp16 cos/sin basis (input-independent module constant),
pre-transposes h, and assembles out = [lo[:, :25129], reverse(hi[:, 1:25129])].
The sqrt(2/V) scale is folded into both W and the basis.
"""

import math
import os
import sys

import numpy as np

for _p in ("/opt/trn_rl_repo", "/root/.axon_site/_ro/trn_rl_repo"):
    if os.path.isdir(_p) and _p not in sys.path:
        sys.path.append(_p)

import concourse.bass as bass
import concourse.tile as tile
from concourse import mybir
from concourse.bass import _add_dep_helper
from concourse.bass_utils import run_bass_kernel_spmd

N_FREQ = 256
V = 50257
C = 1024
B, S = 4, 1024
ROWS = B * S            # 4096
N_CORES = 8
RPC = ROWS // N_CORES   # 512 rows per core
T_HALF = V // 2 + 1     # 25129 (half-spectrum length, V odd)
NT = 512                # t-chunk width (one PSUM bank of fp32)
NCHUNK = (T_HALF + NT - 1) // NT   # 50
T_PAD = NCHUNK * NT     # 25600 (pad columns computed then dropped on host)

F16 = mybir.dt.float16
F32 = mybir.dt.float32

# Output DRAM dtype: fp16 halves the dominant output-write DMA traffic; the
# host upcasts to fp32. Set KERNEL_OUT_F32=1 to fall back to fp32 outputs.
OUT_F32 = bool(int(os.environ.get("KERNEL_OUT_F32", "0")))
OUT_DT = F32 if OUT_F32 else F16
OUT_NP = np.float32 if OUT_F32 else np.float16

# Stash of the last device-run results so test.py can read exec_time_ns.
LAST_RESULTS = None

_BASIS_CACHE = {}


def _make_basis() -> np.ndarray:
    """[2n, T_PAD] fp16: rows 0..n-1 = scale*cos, rows n..2n-1 = scale*sin."""
    if "b" not in _BASIS_CACHE:
        scale = math.sqrt(2.0 / V)
        k = np.arange(1, N_FREQ + 1, dtype=np.float64)[:, None]
        t = np.arange(T_PAD, dtype=np.float64)[None, :]
        ang = (2.0 * np.pi / V) * (k * t)
        _BASIS_CACHE["b"] = np.concatenate(
            [scale * np.cos(ang), scale * np.sin(ang)], axis=0
        ).astype(np.float16)
    return _BASIS_CACHE["b"]


def _build_nc() -> bass.Bass:
    nc = bass.Bass(trn_type="TRN2")

    ht = nc.dram_tensor("ht", [C, RPC], F16, kind="ExternalInput")
    w = nc.dram_tensor("w", [C, 2 * N_FREQ], F16, kind="ExternalInput")
    basis = nc.dram_tensor("basis", [2 * N_FREQ, T_PAD], F16, kind="ExternalInput")
    out_lo = nc.dram_tensor("out_lo", [RPC, T_PAD], OUT_DT, kind="ExternalOutput")
    out_hi = nc.dram_tensor("out_hi", [RPC, T_PAD], OUT_DT, kind="ExternalOutput")

    ht_r = ht[:, :].rearrange("(k p) r -> p k r", p=128)       # [128, 8, 512]
    w_r = w[:, :].rearrange("(k p) f -> p k f", p=128)         # [128, 8, 512]
    basis_r = basis[:, :].rearrange("(j p) t -> p j t", p=128)  # [128, 4, T_PAD]

    with tile.TileContext(nc) as tc:
        with (
            tc.tile_pool(name="singles", bufs=1) as singles,
            tc.tile_pool(name="bpool", bufs=3) as bpool,
            tc.tile_pool(name="opool", bufs=8) as opool,
            tc.tile_pool(name="cpool", bufs=6) as cpool,
            tc.tile_pool(name="psum1", bufs=1, space="PSUM") as psum1,
            tc.tile_pool(name="psum2", bufs=3, space="PSUM") as psum2,
            tc.tile_pool(name="psumd", bufs=1, space="PSUM") as psumd,
        ):
            ht_sb = singles.tile([128, 8, RPC], F16)
            nc.sync.dma_start(out=ht_sb, in_=ht_r)
            w_sb = singles.tile([128, 8, 2 * N_FREQ], F16)
            nc.sync.dma_start(out=w_sb, in_=w_r)

            # stage 1: Y^T [512 f, RPC rows] as 4 f-tiles of [128, RPC]
            y_sb = singles.tile([128, 4, RPC], F16)
            for jf in range(4):
                py = psum1.tile([128, RPC], F32, tag="py")
                for k in range(8):
                    nc.tensor.matmul(
                        py,
                        w_sb[:, k, jf * 128:(jf + 1) * 128],
                        ht_sb[:, k, :],
                        start=(k == 0),
                        stop=(k == 7),
                    )
                nc.scalar.copy(out=y_sb[:, jf, :], in_=py)

            # stage 2
            for g in range(NCHUNK):
                b_sb = bpool.tile([128, 4, NT], F16, tag="b")
                nc.sync.dma_start(out=b_sb, in_=basis_r[:, :, g * NT:(g + 1) * NT])
                for r in range(4):
                    rs = slice(r * 128, (r + 1) * 128)
                    # one PSUM tile spanning two adjacent banks: bank 0 = C,
                    # bank 1 = S; downstream engines read it with one copy.
                    pcs = psum2.tile([128, 2, NT], F32, tag="pcs")
                    nc.tensor.matmul(pcs[:, 0, :], y_sb[:, 0, rs], b_sb[:, 0, :], start=True, stop=False)
                    nc.tensor.matmul(pcs[:, 0, :], y_sb[:, 1, rs], b_sb[:, 1, :], start=False, stop=True)
                    nc.tensor.matmul(pcs[:, 1, :], y_sb[:, 2, rs], b_sb[:, 2, :], start=True, stop=False)
                    nc.tensor.matmul(pcs[:, 1, :], y_sb[:, 3, rs], b_sb[:, 3, :], start=False, stop=True)

                    lo = opool.tile([128, NT], OUT_DT, tag="lo")
                    hi = opool.tile([128, NT], OUT_DT, tag="hi")
                    cs = cpool.tile([128, 2, NT], F16, tag="cs")
                    if r < 3:
                        # ScalarE moves psum->sbuf (fp16); VectorE combines in
                        # its 16-bit SBUF mode.
                        nc.scalar.copy(out=cs, in_=pcs)
                    else:
                        # Spread the psum reads: this tile's copy runs on
                        # VectorE instead of ScalarE.
                        nc.vector.tensor_copy(out=cs, in_=pcs)
                    nc.vector.tensor_sub(lo, cs[:, 0, :], cs[:, 1, :])
                    nc.vector.tensor_add(hi, cs[:, 0, :], cs[:, 1, :])
                    nc.sync.dma_start(
                        out=out_lo[rs, g * NT:(g + 1) * NT], in_=lo
                    )
                    nc.sync.dma_start(
                        out=out_hi[rs, g * NT:(g + 1) * NT], in_=hi
                    )

    _hoist_excess_waits(nc)
    return nc


def _hoist_excess_waits(nc: bass.Bass) -> int:
    """Walrus encodes at most ONE sync-wait on TPB compute instructions
    (matmul / tensor_tensor / activation / ...). Tile freely emits 2-3.
    Hoist the excess onto standalone InstEventSemaphore carriers (pure
    sequencer wait ops, same engine, immediately before the instruction)."""
    import bass_rust

    split_types = {
        "InstMatmult", "InstLdweights", "InstTensorTensor", "InstTensorCopy",
        "InstActivation", "InstMemset", "InstTensorScalar", "InstIota",
        "InstTensorReduce", "InstDMACopy", "InstDrain",
    }
    n = 0
    fn = list(nc.m.functions)[0]
    for blk in list(fn.blocks):
        insts = list(blk.instructions)
        out = []
        changed = False
        for i in insts:
            si = i.sync_info
            if (
                si is not None
                and type(i).__name__ in split_types
                and len(si.on_wait) > 1
            ):
                waits = list(si.on_wait)
                for w in waits[:-1]:
                    out.append(bass_rust.InstEventSemaphore(
                        name=f"wsplit_{n}",
                        engine=i.engine,
                        ins=[],
                        outs=[],
                        sync_info=bass_rust.SyncInfo(on_wait=[w], on_update=[]),
                    ))
                    n += 1
                i.sync_info = bass_rust.SyncInfo(
                    on_wait=waits[-1:], on_update=list(si.on_update)
                )
                changed = True
            out.append(i)
        if changed:
            blk.instructions = out
    return n


def kernel(h: np.ndarray, weight: np.ndarray) -> np.ndarray:
    global LAST_RESULTS
    h = np.asarray(h)
    weight = np.asarray(weight)
    scale = math.sqrt(2.0 / V)

    ht = np.ascontiguousarray(h.reshape(ROWS, C).T.astype(np.float16))  # [C, ROWS]
    w16 = (weight.astype(np.float64) * scale).astype(np.float16)        # [C, 2n]
    basis = _make_basis()

    in_maps = []
    for c in range(N_CORES):
        in_maps.append({
            "ht": np.ascontiguousarray(ht[:, c * RPC:(c + 1) * RPC]),
            "w": w16,
            "basis": basis,
        })

    nc = _build_nc()
    res = run_bass_kernel_spmd(
        nc,
        in_maps,
        core_ids=list(range(N_CORES)),
        trace=bool(int(os.environ.get("KERNEL_TRACE", "0"))),
    )
    LAST_RESULTS = res

    out = np.empty((ROWS, V), dtype=np.float32)
    for c in range(N_CORES):
        lo = res.results[c]["out_lo"]
        hi = res.results[c]["out_hi"]
        rows = slice(c * RPC, (c + 1) * RPC)
        out[rows, :T_HALF] = lo[:, :T_HALF].astype(np.float32)
        out[rows, T_HALF:] = hi[:, 1:T_HALF][:, ::-1].astype(np.float32)
    return out.reshape(B, S, V)


# revision 13
# speedup vs baseline: 1.5241x; 1.5241x over previous
"""GaussSynthesis Trainium2 kernel.

reference:  Y_ri = h @ weight            [B,S,2n]  (n=256 freqs)
            full spectrum bins 1..n = Y, rest zero
            out  = irfft(full, n=V)      [B,S,V]   (V=50257, odd)

Closed form (V odd, only bins 1..n nonzero):
    out[t]   = (2/V) * sum_k ( R_k cos(2 pi k t / V) - I_k sin(2 pi k t / V) )
    out[V-t] = (2/V) * sum_k ( R_k cos(2 pi k t / V) + I_k sin(2 pi k t / V) )
so only t = 0..(V-1)/2 = 25128 must be computed: two matmuls against a
cos/sin basis, then a sum/difference combine covers the full output.

Device plan (SPMD over 8 cores, 512 rows each, no collectives):
  stage 1: Y^T[f, r] = (scale*W)^T @ h^T   (fp16 inputs, fp32 psum -> fp16)
  stage 2: per 512-wide t-chunk: psum_c = R^T-part @ cos-chunk,
           psum_s = I^T-part @ sin-chunk (2 accumulating matmuls each),
           lo = c - s, hi = c + s  (ScalarE copies + VectorE tensor_tensor),
           DMA lo/hi to DRAM.
Host: builds the fp16 cos/sin basis (input-independent module constant),
pre-transposes h, and assembles out = [lo[:, :25129], reverse(hi[:, 1:25129])].
The sqrt(2/V) scale is folded into both W and the basis.
"""

import math
import os
import sys

import numpy as np

for _p in ("/opt/trn_rl_repo", "/root/.axon_site/_ro/trn_rl_repo"):
    if os.path.isdir(_p) and _p not in sys.path:
        sys.path.append(_p)

import concourse.bass as bass
import concourse.tile as tile
from concourse import mybir
from concourse.bass import _add_dep_helper
from concourse.bass_utils import run_bass_kernel_spmd

N_FREQ = 256
V = 50257
C = 1024
B, S = 4, 1024
ROWS = B * S            # 4096
N_CORES = 8
RPC = ROWS // N_CORES   # 512 rows per core
T_HALF = V // 2 + 1     # 25129 (half-spectrum length, V odd)
NT = 512                # t-chunk width (one PSUM bank of fp32)
NCHUNK = (T_HALF + NT - 1) // NT   # 50
T_PAD = NCHUNK * NT     # 25600 (pad columns computed then dropped on host)

F16 = mybir.dt.float16
F32 = mybir.dt.float32

# Output DRAM dtype: fp16 halves the dominant output-write DMA traffic; the
# host upcasts to fp32. Set KERNEL_OUT_F32=1 to fall back to fp32 outputs.
OUT_F32 = bool(int(os.environ.get("KERNEL_OUT_F32", "0")))
OUT_DT = F32 if OUT_F32 else F16
OUT_NP = np.float32 if OUT_F32 else np.float16

# Stash of the last device-run results so test.py can read exec_time_ns.
LAST_RESULTS = None

_BASIS_CACHE = {}


def _make_basis() -> np.ndarray:
    """[2n, T_PAD] fp16: rows 0..n-1 = scale*cos, rows n..2n-1 = scale*sin."""
    if "b" not in _BASIS_CACHE:
        scale = math.sqrt(2.0 / V)
        k = np.arange(1, N_FREQ + 1, dtype=np.float64)[:, None]
        t = np.arange(T_PAD, dtype=np.float64)[None, :]
        ang = (2.0 * np.pi / V) * (k * t)
        _BASIS_CACHE["b"] = np.concatenate(
            [scale * np.cos(ang), scale * np.sin(ang)], axis=0
        ).astype(np.float16)
    return _BASIS_CACHE["b"]


def _build_nc() -> bass.Bass:
    nc = bass.Bass(trn_type="TRN2")

    ht = nc.dram_tensor("ht", [C, RPC], F16, kind="ExternalInput")
    w = nc.dram_tensor("w", [C, 2 * N_FREQ], F16, kind="ExternalInput")
    basis = nc.dram_tensor("basis", [2 * N_FREQ, T_PAD], F16, kind="ExternalInput")
    out_lo = nc.dram_tensor("out_lo", [RPC, T_PAD], OUT_DT, kind="ExternalOutput")
    out_hi = nc.dram_tensor("out_hi", [RPC, T_PAD], OUT_DT, kind="ExternalOutput")

    ht_r = ht[:, :].rearrange("(k p) r -> p k r", p=128)       # [128, 8, 512]
    w_r = w[:, :].rearrange("(k p) f -> p k f", p=128)         # [128, 8, 512]
    basis_r = basis[:, :].rearrange("(j p) t -> p j t", p=128)  # [128, 4, T_PAD]

    with tile.TileContext(nc) as tc:
        with (
            tc.tile_pool(name="singles", bufs=1) as singles,
            tc.tile_pool(name="bpool", bufs=3) as bpool,
            tc.tile_pool(name="opool", bufs=8) as opool,
            tc.tile_pool(name="cpool", bufs=6) as cpool,
            tc.tile_pool(name="psum1", bufs=1, space="PSUM") as psum1,
            tc.tile_pool(name="psum2", bufs=3, space="PSUM") as psum2,
            tc.tile_pool(name="psumd", bufs=1, space="PSUM") as psumd,
        ):
            ht_sb = singles.tile([128, 8, RPC], F16)
            nc.sync.dma_start(out=ht_sb, in_=ht_r)
            w_sb = singles.tile([128, 8, 2 * N_FREQ], F16)
            nc.sync.dma_start(out=w_sb, in_=w_r)

            # stage 1: Y^T [512 f, RPC rows] as 4 f-tiles of [128, RPC]
            y_sb = singles.tile([128, 4, RPC], F16)
            for jf in range(4):
                py = psum1.tile([128, RPC], F32, tag="py")
                for k in range(8):
                    nc.tensor.matmul(
                        py,
                        w_sb[:, k, jf * 128:(jf + 1) * 128],
                        ht_sb[:, k, :],
                        start=(k == 0),
                        stop=(k == 7),
                    )
                nc.scalar.copy(out=y_sb[:, jf, :], in_=py)

            # stage 2 — chunk PAIRS: one basis load and one lo/hi store per
            # pair, so DMA partition lines are 2 KB and the Sync queue sees
            # half the entries (each out-DMA wait head-of-line-blocks it).
            for gp in range(NCHUNK // 2):
                g0 = 2 * gp
                b_sb = bpool.tile([128, 4, 2 * NT], F16, tag="b")
                nc.sync.dma_start(
                    out=b_sb, in_=basis_r[:, :, g0 * NT:(g0 + 2) * NT]
                )
                for r in range(4):
                    rs = slice(r * 128, (r + 1) * 128)
                    lo = opool.tile([128, 2, NT], OUT_DT, tag="lo")
                    hi = opool.tile([128, 2, NT], OUT_DT, tag="hi")
                    for gg in range(2):
                        # one PSUM tile spanning two adjacent banks: bank 0 =
                        # C, bank 1 = S; downstream reads it with one copy.
                        bs = slice(gg * NT, (gg + 1) * NT)
                        pcs = psum2.tile([128, 2, NT], F32, tag="pcs")
                        nc.tensor.matmul(pcs[:, 0, :], y_sb[:, 0, rs], b_sb[:, 0, bs], start=True, stop=False)
                        nc.tensor.matmul(pcs[:, 0, :], y_sb[:, 1, rs], b_sb[:, 1, bs], start=False, stop=True)
                        nc.tensor.matmul(pcs[:, 1, :], y_sb[:, 2, rs], b_sb[:, 2, bs], start=True, stop=False)
                        nc.tensor.matmul(pcs[:, 1, :], y_sb[:, 3, rs], b_sb[:, 3, bs], start=False, stop=True)

                        cs = cpool.tile([128, 2, NT], F16, tag="cs")
                        if r < 3:
                            # ScalarE moves psum->sbuf (fp16); VectorE
                            # combines in its 16-bit SBUF mode.
                            nc.scalar.copy(out=cs, in_=pcs)
                        else:
                            # Spread the psum reads: this tile's copy runs
                            # on VectorE instead of ScalarE.
                            nc.vector.tensor_copy(out=cs, in_=pcs)
                        nc.vector.tensor_sub(lo[:, gg, :], cs[:, 0, :], cs[:, 1, :])
                        nc.vector.tensor_add(hi[:, gg, :], cs[:, 0, :], cs[:, 1, :])
                    nc.sync.dma_start(
                        out=out_lo[rs, g0 * NT:(g0 + 2) * NT], in_=lo
                    )
                    nc.sync.dma_start(
                        out=out_hi[rs, g0 * NT:(g0 + 2) * NT], in_=hi
                    )

    _hoist_excess_waits(nc)
    return nc


def _hoist_excess_waits(nc: bass.Bass) -> int:
    """Walrus encodes at most ONE sync-wait on TPB compute instructions
    (matmul / tensor_tensor / activation / ...). Tile freely emits 2-3.
    Hoist the excess onto standalone InstEventSemaphore carriers (pure
    sequencer wait ops, same engine, immediately before the instruction)."""
    import bass_rust

    split_types = {
        "InstMatmult", "InstLdweights", "InstTensorTensor", "InstTensorCopy",
        "InstActivation", "InstMemset", "InstTensorScalar", "InstIota",
        "InstTensorReduce", "InstDMACopy", "InstDrain",
    }
    n = 0
    fn = list(nc.m.functions)[0]
    for blk in list(fn.blocks):
        insts = list(blk.instructions)
        out = []
        changed = False
        for i in insts:
            si = i.sync_info
            if (
                si is not None
                and type(i).__name__ in split_types
                and len(si.on_wait) > 1
            ):
                waits = list(si.on_wait)
                for w in waits[:-1]:
                    out.append(bass_rust.InstEventSemaphore(
                        name=f"wsplit_{n}",
                        engine=i.engine,
                        ins=[],
                        outs=[],
                        sync_info=bass_rust.SyncInfo(on_wait=[w], on_update=[]),
                    ))
                    n += 1
                i.sync_info = bass_rust.SyncInfo(
                    on_wait=waits[-1:], on_update=list(si.on_update)
                )
                changed = True
            out.append(i)
        if changed:
            blk.instructions = out
    return n


def kernel(h: np.ndarray, weight: np.ndarray) -> np.ndarray:
    global LAST_RESULTS
    h = np.asarray(h)
    weight = np.asarray(weight)
    scale = math.sqrt(2.0 / V)

    ht = np.ascontiguousarray(h.reshape(ROWS, C).T.astype(np.float16))  # [C, ROWS]
    w16 = (weight.astype(np.float64) * scale).astype(np.float16)        # [C, 2n]
    basis = _make_basis()

    in_maps = []
    for c in range(N_CORES):
        in_maps.append({
            "ht": np.ascontiguousarray(ht[:, c * RPC:(c + 1) * RPC]),
            "w": w16,
            "basis": basis,
        })

    nc = _build_nc()
    res = run_bass_kernel_spmd(
        nc,
        in_maps,
        core_ids=list(range(N_CORES)),
        trace=bool(int(os.environ.get("KERNEL_TRACE", "0"))),
    )
    LAST_RESULTS = res

    out = np.empty((ROWS, V), dtype=np.float32)
    for c in range(N_CORES):
        lo = res.results[c]["out_lo"]
        hi = res.results[c]["out_hi"]
        rows = slice(c * RPC, (c + 1) * RPC)
        out[rows, :T_HALF] = lo[:, :T_HALF].astype(np.float32)
        out[rows, T_HALF:] = hi[:, 1:T_HALF][:, ::-1].astype(np.float32)
    return out.reshape(B, S, V)


# revision 16
# speedup vs baseline: 1.5713x; 1.0310x over previous
"""GaussSynthesis Trainium2 kernel.

reference:  Y_ri = h @ weight            [B,S,2n]  (n=256 freqs)
            full spectrum bins 1..n = Y, rest zero
            out  = irfft(full, n=V)      [B,S,V]   (V=50257, odd)

Closed form (V odd, only bins 1..n nonzero):
    out[t]   = (2/V) * sum_k ( R_k cos(2 pi k t / V) - I_k sin(2 pi k t / V) )
    out[V-t] = (2/V) * sum_k ( R_k cos(2 pi k t / V) + I_k sin(2 pi k t / V) )
so only t = 0..(V-1)/2 = 25128 must be computed: two matmuls against a
cos/sin basis, then a sum/difference combine covers the full output.

Device plan (SPMD over 8 cores, 512 rows each, no collectives):
  stage 1: Y^T[f, r] = (scale*W)^T @ h^T   (fp16 inputs, fp32 psum -> fp16)
  stage 2: per 512-wide t-chunk: psum_c = R^T-part @ cos-chunk,
           psum_s = I^T-part @ sin-chunk (2 accumulating matmuls each),
           lo = c - s, hi = c + s  (ScalarE copies + VectorE tensor_tensor),
           DMA lo/hi to DRAM.
Host: builds the fp16 cos/sin basis (input-independent module constant),
pre-transposes h, and assembles out = [lo[:, :25129], reverse(hi[:, 1:25129])].
The sqrt(2/V) scale is folded into both W and the basis.
"""

import math
import os
import sys

import numpy as np

for _p in ("/opt/trn_rl_repo", "/root/.axon_site/_ro/trn_rl_repo"):
    if os.path.isdir(_p) and _p not in sys.path:
        sys.path.append(_p)

import concourse.bass as bass
import concourse.tile as tile
from concourse import mybir
from concourse.bass import _add_dep_helper
from concourse.bass_utils import run_bass_kernel_spmd

N_FREQ = 256
V = 50257
C = 1024
B, S = 4, 1024
ROWS = B * S            # 4096
N_CORES = 8
RPC = ROWS // N_CORES   # 512 rows per core
T_HALF = V // 2 + 1     # 25129 (half-spectrum length, V odd)
NT = 512                # t-chunk width (one PSUM bank of fp32)
NCHUNK = (T_HALF + NT - 1) // NT   # 50
T_PAD = NCHUNK * NT     # 25600 (pad columns computed then dropped on host)

F16 = mybir.dt.float16
F32 = mybir.dt.float32

# Output DRAM dtype: fp16 halves the dominant output-write DMA traffic; the
# host upcasts to fp32. Set KERNEL_OUT_F32=1 to fall back to fp32 outputs.
OUT_F32 = bool(int(os.environ.get("KERNEL_OUT_F32", "0")))
OUT_DT = F32 if OUT_F32 else F16
OUT_NP = np.float32 if OUT_F32 else np.float16

# Stash of the last device-run results so test.py can read exec_time_ns.
LAST_RESULTS = None

_BASIS_CACHE = {}


def _make_basis() -> np.ndarray:
    """[2n, T_PAD] fp16: rows 0..n-1 = scale*cos, rows n..2n-1 = scale*sin."""
    if "b" not in _BASIS_CACHE:
        scale = math.sqrt(2.0 / V)
        k = np.arange(1, N_FREQ + 1, dtype=np.float64)[:, None]
        t = np.arange(T_PAD, dtype=np.float64)[None, :]
        ang = (2.0 * np.pi / V) * (k * t)
        _BASIS_CACHE["b"] = np.concatenate(
            [scale * np.cos(ang), scale * np.sin(ang)], axis=0
        ).astype(np.float16)
    return _BASIS_CACHE["b"]


def _build_nc() -> bass.Bass:
    nc = bass.Bass(trn_type="TRN2")

    ht = nc.dram_tensor("ht", [C, RPC], F16, kind="ExternalInput")
    w = nc.dram_tensor("w", [C, 2 * N_FREQ], F16, kind="ExternalInput")
    basis = nc.dram_tensor("basis", [2 * N_FREQ, T_PAD], F16, kind="ExternalInput")
    out_lo = nc.dram_tensor("out_lo", [RPC, T_PAD], OUT_DT, kind="ExternalOutput")
    out_hi = nc.dram_tensor("out_hi", [RPC, T_PAD], OUT_DT, kind="ExternalOutput")

    ht_r = ht[:, :].rearrange("(k p) r -> p k r", p=128)       # [128, 8, 512]
    w_r = w[:, :].rearrange("(k p) f -> p k f", p=128)         # [128, 8, 512]
    basis_r = basis[:, :].rearrange("(j p) t -> p j t", p=128)  # [128, 4, T_PAD]

    with tile.TileContext(nc) as tc:
        with (
            tc.tile_pool(name="singles", bufs=1) as singles,
            tc.tile_pool(name="bpool", bufs=2) as bpool,
            tc.tile_pool(name="opool", bufs=4) as opool,
            tc.tile_pool(name="cpool", bufs=6) as cpool,
            tc.tile_pool(name="psum1", bufs=1, space="PSUM") as psum1,
            tc.tile_pool(name="psum2", bufs=3, space="PSUM") as psum2,
            tc.tile_pool(name="psumd", bufs=1, space="PSUM") as psumd,
        ):
            ht_sb = singles.tile([128, 8, RPC], F16)
            nc.sync.dma_start(out=ht_sb, in_=ht_r)
            w_sb = singles.tile([128, 8, 2 * N_FREQ], F16)
            nc.sync.dma_start(out=w_sb, in_=w_r)

            # stage 1: Y^T [512 f, RPC rows] as 4 f-tiles of [128, RPC]
            y_sb = singles.tile([128, 4, RPC], F16)
            for jf in range(4):
                py = psum1.tile([128, RPC], F32, tag="py")
                for k in range(8):
                    nc.tensor.matmul(
                        py,
                        w_sb[:, k, jf * 128:(jf + 1) * 128],
                        ht_sb[:, k, :],
                        start=(k == 0),
                        stop=(k == 7),
                    )
                nc.scalar.copy(out=y_sb[:, jf, :], in_=py)

            # stage 2 — chunk QUADS: one basis load and one lo/hi store per
            # group of 4 chunks, so DMA partition lines are 4 KB and the Sync
            # queue sees few entries (each out-DMA wait head-of-line-blocks
            # it). 50 chunks = 12 quads + 1 tail pair.
            groups = [(4 * q, 4) for q in range(NCHUNK // 4)]
            if NCHUNK % 4:
                groups.append((NCHUNK - NCHUNK % 4, NCHUNK % 4))
            for g0, gw in groups:
                b_sb = bpool.tile([128, 4, gw * NT], F16, tag="b")
                nc.sync.dma_start(
                    out=b_sb, in_=basis_r[:, :, g0 * NT:(g0 + gw) * NT]
                )
                for r in range(4):
                    rs = slice(r * 128, (r + 1) * 128)
                    lo = opool.tile([128, gw, NT], OUT_DT, tag="lo")
                    hi = opool.tile([128, gw, NT], OUT_DT, tag="hi")
                    for gg in range(gw):
                        # one PSUM tile spanning two adjacent banks: bank 0 =
                        # C, bank 1 = S; downstream reads it with one copy.
                        bs = slice(gg * NT, (gg + 1) * NT)
                        pcs = psum2.tile([128, 2, NT], F32, tag="pcs")
                        nc.tensor.matmul(pcs[:, 0, :], y_sb[:, 0, rs], b_sb[:, 0, bs], start=True, stop=False)
                        nc.tensor.matmul(pcs[:, 0, :], y_sb[:, 1, rs], b_sb[:, 1, bs], start=False, stop=True)
                        nc.tensor.matmul(pcs[:, 1, :], y_sb[:, 2, rs], b_sb[:, 2, bs], start=True, stop=False)
                        nc.tensor.matmul(pcs[:, 1, :], y_sb[:, 3, rs], b_sb[:, 3, bs], start=False, stop=True)

                        cs = cpool.tile([128, 2, NT], F16, tag="cs")
                        if r < 3:
                            # ScalarE moves psum->sbuf (fp16); VectorE
                            # combines in its 16-bit SBUF mode.
                            nc.scalar.copy(out=cs, in_=pcs)
                        else:
                            # Spread the psum reads: this tile's copy runs
                            # on VectorE instead of ScalarE.
                            nc.vector.tensor_copy(out=cs, in_=pcs)
                        nc.vector.tensor_sub(lo[:, gg, :], cs[:, 0, :], cs[:, 1, :])
                        nc.vector.tensor_add(hi[:, gg, :], cs[:, 0, :], cs[:, 1, :])
                    nc.sync.dma_start(
                        out=out_lo[rs, g0 * NT:(g0 + gw) * NT], in_=lo
                    )
                    nc.sync.dma_start(
                        out=out_hi[rs, g0 * NT:(g0 + gw) * NT], in_=hi
                    )

    _hoist_excess_waits(nc)
    return nc


def _hoist_excess_waits(nc: bass.Bass) -> int:
    """Walrus encodes at most ONE sync-wait on TPB compute instructions
    (matmul / tensor_tensor / activation / ...). Tile freely emits 2-3.
    Hoist the excess onto standalone InstEventSemaphore carriers (pure
    sequencer wait ops, same engine, immediately before the instruction)."""
    import bass_rust

    split_types = {
        "InstMatmult", "InstLdweights", "InstTensorTensor", "InstTensorCopy",
        "InstActivation", "InstMemset", "InstTensorScalar", "InstIota",
        "InstTensorReduce", "InstDMACopy", "InstDrain",
    }
    n = 0
    fn = list(nc.m.functions)[0]
    for blk in list(fn.blocks):
        insts = list(blk.instructions)
        out = []
        changed = False
        for i in insts:
            si = i.sync_info
            if (
                si is not None
                and type(i).__name__ in split_types
                and len(si.on_wait) > 1
            ):
                waits = list(si.on_wait)
                for w in waits[:-1]:
                    out.append(bass_rust.InstEventSemaphore(
                        name=f"wsplit_{n}",
                        engine=i.engine,
                        ins=[],
                        outs=[],
                        sync_info=bass_rust.SyncInfo(on_wait=[w], on_update=[]),
                    ))
                    n += 1
                i.sync_info = bass_rust.SyncInfo(
                    on_wait=waits[-1:], on_update=list(si.on_update)
                )
                changed = True
            out.append(i)
        if changed:
            blk.instructions = out
    return n


def kernel(h: np.ndarray, weight: np.ndarray) -> np.ndarray:
    global LAST_RESULTS
    h = np.asarray(h)
    weight = np.asarray(weight)
    scale = math.sqrt(2.0 / V)

    ht = np.ascontiguousarray(h.reshape(ROWS, C).T.astype(np.float16))  # [C, ROWS]
    w16 = (weight.astype(np.float64) * scale).astype(np.float16)        # [C, 2n]
    basis = _make_basis()

    in_maps = []
    for c in range(N_CORES):
        in_maps.append({
            "ht": np.ascontiguousarray(ht[:, c * RPC:(c + 1) * RPC]),
            "w": w16,
            "basis": basis,
        })

    nc = _build_nc()
    res = run_bass_kernel_spmd(
        nc,
        in_maps,
        core_ids=list(range(N_CORES)),
        trace=bool(int(os.environ.get("KERNEL_TRACE", "0"))),
    )
    LAST_RESULTS = res

    out = np.empty((ROWS, V), dtype=np.float32)
    for c in range(N_CORES):
        lo = res.results[c]["out_lo"]
        hi = res.results[c]["out_hi"]
        rows = slice(c * RPC, (c + 1) * RPC)
        out[rows, :T_HALF] = lo[:, :T_HALF].astype(np.float32)
        out[rows, T_HALF:] = hi[:, 1:T_HALF][:, ::-1].astype(np.float32)
    return out.reshape(B, S, V)
